# revision 1
# baseline (speedup 1.0000x reference)
"""Trainium2 Bass kernel for nn_BinaryTTN (batch 2048, 12-layer binary tree
tensor network), data-parallel across 8 NeuronCores.

Key structure (per core, n=256 samples, fp16 state, fp32 PSUM accum):
  * Layers 0+1 folded on host into layer-2 weights: with EMBED=2 the output
    of an L1 subtree is linear in the 16 monomials z4 = outer product of the
    four input 2-vectors of its 2x2 patch.
  * z-stage: z0 = xt*xb on DVE, PE ones-matmuls broadcast/permute rows,
    z4 = z0L*z0R on DVE.
  * Generic layer at location p: t = Wfold^T l (PE, K=16, 32-row-tiled),
    ACT evacuates t (fp32 PSUM -> fp16 SBUF), DVE multiplies by the right
    child's REP-form output (8x replicated rows, broadcast over chunks),
    PE block-diagonal ones-matmul reduces over j writing the REP-form
    output for the next layer.
All PE ops use tile_position with K<=32 so the array stays in 4x row-tiled
mode (no mode-switch drains).
"""
import sys
import numpy as np

sys.path.insert(0, '/opt/trn_rl_repo')

BATCH, EMBED, H0, W0 = 2048, 2, 64, 64
NCORES = 8
NSH = BATCH // NCORES      # 256
NBLK = 8                   # c1-blocks of 4 (blocks of 8 image columns)


def layer_specs():
    out = []
    H, W, ind = H0, W0, EMBED
    for li in range(12):
        bond = 1 if li == 11 else 16
        o = H < W
        h = H // (1 if o else 2)
        w = W // (2 if o else 1)
        out.append((h, w, bond, ind, o))
        H, W, ind = h, w, bond
    return out


SPECS = layer_specs()


def block_locs(li, blk):
    h, w = SPECS[li][0], SPECS[li][1]
    frac = w / 32.0
    x0, x1 = int(4 * blk * frac), int(4 * (blk + 1) * frac)
    return [(y, x) for y in range(h) for x in range(x0, x1)]


def _build_structure():
    """Static packing structure: loc -> (blk, a, hbm col); slabs."""
    loc_tab, slabs, shapes = {}, {}, {}
    for li in range(2, 11):
        off = 0
        blks = range(NBLK) if li <= 6 else [0]
        for blk in blks:
            locs = (block_locs(li, blk) if li <= 6 else
                    [(y, x) for y in range(SPECS[li][0]) for x in range(SPECS[li][1])])
            groups = [[] for _ in range(4)]
            for i, (y, x) in enumerate(locs):
                a = (y % 4) if li == 2 else (i % 4)
                groups[a].append((y, x))
            for a in range(4):
                start = off
                for (y, x) in groups[a]:
                    loc_tab[(li, y, x)] = (blk, a, off)
                    off += 256
                slabs[(li, blk, a)] = (start, off - start)
        shapes[li] = off
    return loc_tab, slabs, shapes


LOC_TAB, SLABS, WHBM_SHAPES = _build_structure()


# ---------------- host packing of values ----------------
def fold_weights(ws):
    W0m = ws[0].reshape(32, 64, 16, 4)
    T1 = np.einsum('rcbij,rcik,rcjl->rcbkl',
                   ws[1], W0m[:, 0::2], W0m[:, 1::2]).reshape(32, 32, 16, 16)
    Q2 = np.einsum('rcbij,rcik,rcjl->rcbkl', ws[2], T1[0::2], T1[1::2])
    folded = {}
    q = Q2.transpose(0, 1, 3, 2, 4).reshape(16, 32, 16, 2, 8, 16)
    folded[2] = q.transpose(0, 1, 3, 2, 4, 5).reshape(16, 32, 2, 16, 128)
    for li in range(3, 11):
        h, w = SPECS[li][0], SPECS[li][1]
        q = ws[li].transpose(0, 1, 3, 2, 4).reshape(h, w, 16, 2, 8, 16)
        folded[li] = q.transpose(0, 1, 3, 2, 4, 5).reshape(h, w, 2, 16, 128)
    folded[11] = ws[11].transpose(0, 1, 3, 2, 4).reshape(16, 16)  # [i, j]
    return folded


def pack_weights(ws):
    folded = fold_weights(ws)
    whbm = {}
    for li in range(2, 11):
        arr = np.zeros((16, WHBM_SHAPES[li]), dtype=np.float16)
        for (lli, y, x), (blk, a, off) in LOC_TAB.items():
            if lli != li:
                continue
            arr[:, off:off + 256] = folded[li][y, x].transpose(1, 0, 2).reshape(16, 256)
        whbm[li] = arr
    whbm[11] = folded[11].astype(np.float16)
    return whbm


def build_consts():
    c = {}
    red = np.zeros((128, 256), dtype=np.float16)
    for ch in range(2):
        for bl in range(8):
            b = ch * 8 + bl
            for k in range(16):
                for s in range(8):
                    red[bl * 16 + k, ch * 128 + s * 16 + b] = 1.0
    c['red'] = red
    repk = np.zeros((128, 128), dtype=np.float16)
    for a in range(4):
        for k in range(16):
            for s in range(8):
                repk[32 * a + k, s * 16 + k] = 1.0
    c['repk'] = repk
    for name, par, sel in (('bce0', 0, 0), ('bce1', 0, 1),
                           ('bco0', 1, 0), ('bco1', 1, 1)):
        m = np.zeros((32, 128), dtype=np.float16)
        for g in range(4):
            r8 = 2 * g + par
            for j in range(16):
                kl, kr = j // 4, j % 4
                m[r8 * 4 + (kl if sel == 0 else kr), 32 * g + j] = 1.0
        c[name] = m
    o = np.zeros((128, 4), dtype=np.float16)
    o[0:16, 0] = 1.0
    c['ones16'] = o
    return c


def pack_x(xsh):
    """xsh [nsh, 2, 64, 64] fp32 -> xt/xb [32, 4*64*nsh] fp16.
    rows (r8, ch, dup): xt val = x[n, i0, 2r, c]; free (rhi, c, n)."""
    n = xsh.shape[0]
    x16 = xsh.astype(np.float16).reshape(n, 2, 32, 2, 64)
    xt = x16[:, :, :, 0].transpose(1, 2, 3, 0).reshape(2, 4, 8, 64, n)
    xb = x16[:, :, :, 1].transpose(1, 2, 3, 0).reshape(2, 4, 8, 64, n)
    out_t = np.empty((8, 2, 2, 4, 64, n), dtype=np.float16)
    out_b = np.empty((8, 2, 2, 4, 64, n), dtype=np.float16)
    for dup in range(2):
        out_t[:, :, dup] = xt.transpose(2, 0, 1, 3, 4)
        out_b[:, dup] = xb.transpose(2, 0, 1, 3, 4)
    return out_t.reshape(32, -1), out_b.reshape(32, -1)


# ---------------- device program ----------------
_PROGRAM = None


def build_program(num_devices=NCORES, dbg=None, maxli=11, wq='gpsimd', nblk=NBLK):
    from contextlib import ExitStack
    import concourse.bass as bass
    import concourse.tile as tile
    from concourse import bacc, mybir

    F16, F32 = mybir.dt.float16, mybir.dt.float32
    n = NSH
    nc = bacc.Bacc("TRN2", target_bir_lowering=False, debug=False,
                   num_devices=num_devices)
    _wdma = None  # set inside context
    xt_h = nc.declare_dram_parameter("xt", [32, 4 * 64 * n], F16, isOutput=False)
    xb_h = nc.declare_dram_parameter("xb", [32, 4 * 64 * n], F16, isOutput=False)
    wh = {li: nc.declare_dram_parameter(f"w{li}", [16, WHBM_SHAPES[li]], F16,
                                        isOutput=False) for li in range(2, 11)}
    wh[11] = nc.declare_dram_parameter("w11", [16, 16], F16, isOutput=False)
    red_h = nc.declare_dram_parameter("red", [128, 256], F16, isOutput=False)
    repk_h = nc.declare_dram_parameter("repk", [128, 128], F16, isOutput=False)
    bc_h = {nm: nc.declare_dram_parameter(nm, [32, 128], F16, isOutput=False)
            for nm in ('bce0', 'bce1', 'bco0', 'bco1')}
    ones_h = nc.declare_dram_parameter("ones16", [128, 4], F16, isOutput=False)
    out_h = nc.declare_dram_parameter("out", [1, n], F32, isOutput=True)
    dbg_h = (nc.declare_dram_parameter("dbg", [128, n], F16, isOutput=True)
             if dbg is not None else None)

    REP_BUFS = {2: 6, 3: 6, 4: 4, 5: 6, 6: 18, 7: 9, 8: 5, 9: 3, 10: 2}

    with tile.TileContext(nc) as tc, ExitStack() as ctx:
        cpool = ctx.enter_context(tc.tile_pool(name="consts", bufs=1))
        red = cpool.tile([128, 256], F16); nc.sync.dma_start(red[:], red_h[:])
        repk = cpool.tile([128, 128], F16); nc.sync.dma_start(repk[:], repk_h[:])
        bct = {}
        for nm in bc_h:
            bct[nm] = cpool.tile([32, 128], F16, tag=f'bc{nm}', name=f'bc{nm}')
            nc.sync.dma_start(bct[nm][:], bc_h[nm][:])
        ones16 = cpool.tile([128, 4], F16); nc.sync.dma_start(ones16[:], ones_h[:])
        w11t = cpool.tile([128, 16], F16); nc.sync.dma_start(w11t[0:16, :], wh[11][:])

        xpool = ctx.enter_context(tc.tile_pool(name="x", bufs=1))
        zpool = ctx.enter_context(tc.tile_pool(name="z", bufs=1))
        z4pool = ctx.enter_context(tc.tile_pool(name="z4", bufs=1))
        zrpool = ctx.enter_context(tc.tile_pool(name="zr", bufs=4))
        wpool = ctx.enter_context(tc.tile_pool(name="w", bufs=2))
        tpool = ctx.enter_context(tc.tile_pool(name="t", bufs=2))
        mpool = ctx.enter_context(tc.tile_pool(name="m", bufs=2))
        rpools = {li: ctx.enter_context(
            tc.tile_pool(name=f"rep{li}", bufs=REP_BUFS[li]))
            for li in range(2, 11)}
        ps_t = ctx.enter_context(tc.tile_pool(name="ps_t", bufs=2, space="PSUM"))
        ps_r = ctx.enter_context(tc.tile_pool(name="ps_r", bufs=2, space="PSUM"))
        ps_z = ctx.enter_context(tc.tile_pool(name="ps_z", bufs=2, space="PSUM"))

        rep = {}
        _evac_tog = [0]

        def evac(dst, src):
            nc.scalar.copy(dst, src)

        def two_step_batch(li, locs, wtile):
            nl = len(locs)
            tp = ps_t.tile([128, 512 * nl], F32, tag="t")
            for i, (y, x, a, woff, lget, rget) in enumerate(locs):
                rhs = lget()
                for c in range(2):
                    nc.tensor.matmul(
                        tp[:, i * 512 + c * 256:i * 512 + (c + 1) * 256],
                        wtile[32 * a:32 * a + 16, woff + c * 128:woff + (c + 1) * 128],
                        rhs, start=True, stop=True, tile_position=(32 * a, 0))
            tsb = tpool.tile([128, 512 * nl], F16, tag="t16")
            evac(tsb[:], tp[:])
            msb = mpool.tile([128, 512 * nl], F16, tag="m16")
            for i, (y, x, a, woff, lget, rget) in enumerate(locs):
                nc.vector.tensor_mul(
                    msb[:, i * 512:(i + 1) * 512].rearrange("p (c nn) -> p c nn", c=2),
                    tsb[:, i * 512:(i + 1) * 512].rearrange("p (c nn) -> p c nn", c=2),
                    rget().unsqueeze(1).broadcast_to([128, 2, n]))
            for i0 in range(0, nl, 2):
                cnt = min(2, nl - i0)
                pr = ps_r.tile([128, 256 * cnt], F32, tag="r")
                for c in range(2):
                    for j in range(cnt):
                        i = i0 + j
                        nc.tensor.matmul(
                            pr[:, j * 256:(j + 1) * 256],
                            red[:, c * 128:(c + 1) * 128],
                            msb[:, i * 512 + c * 256:i * 512 + (c + 1) * 256],
                            start=(c == 0), stop=(c == 1))
                rsb = rpools[li].tile([128, 256 * cnt], F16, tag=f"rep{li}")
                evac(rsb[:], pr[:])
                for j in range(cnt):
                    rep[(li, locs[i0 + j][0], locs[i0 + j][1])] = \
                        rsb[:, j * 256:(j + 1) * 256]

        z4c = {}
        for blk in range(nblk):
            xt = xpool.tile([32, 4 * 8 * n], F16, tag="xt")
            xb = xpool.tile([32, 4 * 8 * n], F16, tag="xb")
            for dst, srch in ((xt, xt_h), (xb, xb_h)):
                nc.sync.dma_start(
                    dst[:].rearrange("p (rhi c nn) -> p rhi c nn", rhi=4, c=8),
                    srch[:].rearrange("p (rhi c nn) -> p rhi c nn", rhi=4, c=64)
                    [:, :, 8 * blk:8 * (blk + 1), :])
            z0 = zpool.tile([32, 4 * 8 * n], F16, tag="z0")
            nc.vector.tensor_mul(z0[:], xt[:], xb[:])
            zin = {}
            for nm in ('bce0', 'bce1', 'bco0', 'bco1'):
                t_ = z4pool.tile([128, 4 * 4 * n], F16, tag=f"zin{nm[-1]}", name=f"zin{nm}")
                par_col = 0 if nm.endswith('0') else 1
                src3 = (z0[:].rearrange("p (rhi c nn) -> p rhi c nn", rhi=4, c=8)
                        [:, :, par_col::2, :])
                dst3 = t_[:].rearrange("p (rhi c nn) -> p rhi c nn", rhi=4, c=4)
                for rhi in range(4):
                    for cp in range(2):
                        pz = ps_z.tile([128, 512], F32, tag="zb", name="pz")
                        nc.tensor.matmul(pz[:], bct[nm][:],
                                         src3[:, rhi, 2 * cp:2 * cp + 2, :],
                                         start=True, stop=True, tile_position=(0, 0))
                        evac(dst3[:, rhi, 2 * cp:2 * cp + 2, :],
                             pz[:].rearrange("p (c nn) -> p c nn", c=2))
                zin[nm] = t_
            for par, i0nm, i1nm in (('E', 'bce0', 'bce1'), ('O', 'bco0', 'bco1')):
                zt = z4pool.tile([128, 4 * 4 * n], F16, tag=f"z4c{par}")
                nc.vector.tensor_mul(zt[:], zin[i0nm][:], zin[i1nm][:])
                z4c[(blk, par)] = zt

            def z4_slice(r1, c1):
                par = 'E' if r1 % 2 == 0 else 'O'
                g = (r1 & 7) // 2
                return (z4c[(blk, par)][32 * g:32 * g + 16, :]
                        .rearrange("p (rhi c nn) -> p rhi c nn", rhi=4, c=4)
                        [:, r1 >> 3, c1 - 4 * blk, :])

            wt = {}
            for li in range(2, min(7, maxli + 1)):
                wcols = max(SLABS[(li, blk, a)][1] for a in range(4)
                            if (li, blk, a) in SLABS)
                wt[li] = wpool.tile([128, wcols], F16, tag=f"w{li}", name=f"wt{li}")
                for a in range(4):
                    if (li, blk, a) not in SLABS or SLABS[(li, blk, a)][1] == 0:
                        continue
                    off, ncol = SLABS[(li, blk, a)]
                    getattr(nc, wq).dma_start(wt[li][32 * a:32 * a + 16, 0:ncol],
                                              wh[li][:, off:off + ncol])

            z4r = {}

            def emit_generic(li, row_locs, wtile):
                orient = SPECS[li][4]
                for i0 in range(0, len(row_locs), 2):
                    locs = []
                    for (y, x) in row_locs[i0:i0 + 2]:
                        blk_, a, off = LOC_TAB[(li, y, x)]
                        woff = off - SLABS[(li, blk, a)][0]
                        cl = (li - 1, y, 2 * x) if orient else (li - 1, 2 * y, x)
                        cr = (li - 1, y, 2 * x + 1) if orient else (li - 1, 2 * y + 1, x)
                        locs.append((y, x, a, woff,
                                     (lambda key=cl, aa=a: rep[key][32 * aa:32 * aa + 16, :]),
                                     (lambda key=cr: rep[key])))
                    two_step_batch(li, locs, wtile)

            def l2row(y2):
                g = ((2 * y2 + 1) & 7) // 2
                for c0 in range(0, 4, 2):
                    pz = ps_z.tile([128, 512], F32, tag="zb", name="pzr")
                    for j in range(2):
                        nc.tensor.matmul(pz[:, j * 256:(j + 1) * 256],
                                         repk[32 * g:32 * g + 16, :],
                                         z4_slice(2 * y2 + 1, 4 * blk + c0 + j),
                                         start=True, stop=True,
                                         tile_position=(32 * g, 0))
                    rt = zrpool.tile([128, 512], F16, tag="z4rep", name="rt")
                    evac(rt[:], pz[:])
                    for j in range(2):
                        z4r[(y2, 4 * blk + c0 + j)] = rt[:, j * 256:(j + 1) * 256]
                for c0 in range(0, 4, 2):
                    locs = []
                    for c1 in range(4 * blk + c0, 4 * blk + c0 + 2):
                        blk_, a, off = LOC_TAB[(2, y2, c1)]
                        woff = off - SLABS[(2, blk, a)][0]
                        locs.append((y2, c1, a, woff,
                                     (lambda r1=2 * y2, c=c1: z4_slice(r1, c)),
                                     (lambda yy=y2, c=c1: z4r[(yy, c)])))
                    two_step_batch(2, locs, wt[2])

            x3s = [x for x in range(2 * blk, 2 * blk + 2)]
            for y2 in range(16 if maxli >= 2 else 0):
                l2row(y2)
                if maxli >= 3:
                    emit_generic(3, [(y2, x) for x in x3s], wt[3])
                if maxli >= 4 and y2 % 2 == 1:
                    y4 = y2 // 2
                    emit_generic(4, [(y4, x) for x in range(2 * blk, 2 * blk + 2)], wt[4])
                    if maxli >= 5:
                        emit_generic(5, [(y4, blk)], wt[5])
                    if maxli >= 6 and y4 == 3:
                        emit_generic(6, [(0, blk), (1, blk)], wt[6])
                    elif maxli >= 6 and y4 == 7:
                        emit_generic(6, [(2, blk), (3, blk)], wt[6])

        for li in range(7, min(11, maxli + 1)):
            h, w, bond, ind, orient = SPECS[li]
            wcols = max(SLABS[(li, 0, a)][1] for a in range(4) if (li, 0, a) in SLABS)
            wtg = wpool.tile([128, wcols], F16, tag="wtail")
            for a in range(4):
                if (li, 0, a) not in SLABS or SLABS[(li, 0, a)][1] == 0:
                    continue
                off, ncol = SLABS[(li, 0, a)]
                getattr(nc, wq).dma_start(wtg[32 * a:32 * a + 16, 0:ncol],
                                          wh[li][:, off:off + ncol])
            locs = []
            for y in range(h):
                for x in range(w):
                    blk_, a, off = LOC_TAB[(li, y, x)]
                    woff = off - SLABS[(li, 0, a)][0]
                    cl = (li - 1, y, 2 * x) if orient else (li - 1, 2 * y, x)
                    cr = (li - 1, y, 2 * x + 1) if orient else (li - 1, 2 * y + 1, x)
                    locs.append((y, x, a, woff,
                                 (lambda key=cl, aa=a: rep[key][32 * aa:32 * aa + 16, :]),
                                 (lambda key=cr: rep[key])))
                    if len(locs) == 2:
                        two_step_batch(li, locs, wtg)
                        locs = []
            if locs:
                two_step_batch(li, locs, wtg)

        if maxli >= 11:
            pt = ps_z.tile([128, 512], F32, tag="zb", name="pt11")
            nc.tensor.matmul(pt[0:16, 0:256], w11t[0:16, :], rep[(10, 0, 0)][0:16, :],
                             start=True, stop=True, tile_position=(0, 0))
            m11 = mpool.tile([16, 256], F16, tag="m11")
            nc.vector.tensor_mul(m11[:], pt[0:16, 0:256], rep[(10, 0, 1)][0:16, :])
            pf = ps_z.tile([128, 512], F32, tag="zb", name="pf")
            nc.tensor.matmul(pf[0:1, 0:256], ones16[0:16, 0:1], m11[:], start=True, stop=True,
                             tile_position=(0, 0))
            osb = tpool.tile([1, 256], F32, tag="outs")
            nc.scalar.copy(osb[:], pf[0:1, 0:256])
            nc.sync.dma_start(out_h[:], osb[:])
        else:
            zz = tpool.tile([1, 256], F32, tag="outs", name="zz")
            nc.any.memset(zz[:], 0.0)
            nc.sync.dma_start(out_h[:], zz[:])
        if dbg is not None:
            dsb = tpool.tile([128, 256], F16, tag="dbg")
            if maxli >= 2:
                nc.vector.tensor_copy(dsb[:], rep[dbg])
            else:
                nc.vector.tensor_copy(dsb[:], z4c[(0, 'E')][:, 0:256])
            nc.sync.dma_start(dbg_h[:], dsb[:])
    nc.compile()
    return nc


def _get_program():
    global _PROGRAM
    if _PROGRAM is None:
        _PROGRAM = build_program()
    return _PROGRAM


def kernel(**inputs):
    from concourse.bass_utils import run_bass_kernel_spmd
    x = np.asarray(inputs['x'])
    ws = [np.asarray(inputs[f'w{i}']) for i in range(12)]
    whbm = pack_weights(ws)
    consts = build_consts()
    nc = _get_program()
    base = {f"w{li}": whbm[li] for li in range(2, 11)}
    base["w11"] = whbm[11]
    base.update(consts)
    in_maps = []
    for core in range(NCORES):
        xt, xb = pack_x(x[core * NSH:(core + 1) * NSH])
        m = dict(base)
        m["xt"] = xt
        m["xb"] = xb
        in_maps.append(m)
    res = run_bass_kernel_spmd(nc, in_maps, list(range(NCORES)))
    out = np.concatenate([res.results[c]["out"].reshape(NSH)
                          for c in range(NCORES)])
    return out.reshape(BATCH, 1, 1, 1).astype(np.float32)

